# revision 46
# baseline (speedup 1.0000x reference)
"""Multi-head self-attention TRN2 kernel (v2).

Full inputs -> shard over 8 NeuronCores as (batch b, head-group g):
core c = 2*b + g handles batch b and heads 8g..8g+7. Each core computes its
heads' contribution to the output projection; the host sums the two partials
per batch and adds proj bias.

v2 structure (per core, T=2048, C=1024, 4 head-pairs, t-chunks of 512):
  - One [128, 1024] PSUM S-window per (st) step holds BOTH heads of the
    pair -> a single EXP instruction per step (ScalarE is the pacer).
  - The two K=64 score matmuls are emitted back-to-back with row groups
    0-63 / 64-127 so they execute concurrently in the PE array.
  - PSUM = 2 double-buffered S-windows (4 banks) + 2 O accumulators
    (2 banks, 65 rows: d 0-63 + rowsum via ones-column of V_aug) + 2
    background banks for the QKV/proj streams. 8 banks exactly.
  - QKV/proj matmuls are "background units" drained one per st step into
    the PE slack under EXP; attention starts as soon as K^T(hp0) and
    Q^T(hp0, th0) exist.
  - Softmax denominators: rowsum rows are bounced to DRAM, regathered as
    [128, 8] (t-major) for one cheap DVE reciprocal, scattered back, then
    stride-0 broadcast to [64, 512] for the normalize multiply.
  - K bias is dropped entirely: adding bk shifts every score column by a
    t-constant, which softmax cancels. Only Q needs its bias.
"""

import numpy as np
import ml_dtypes
from collections import deque
from contextlib import ExitStack

import concourse.bass as bass
import concourse.bacc as bacc
import concourse.tile as tile
from concourse import mybir
from concourse.bass_utils import run_bass_kernel_spmd

BF16 = mybir.dt.bfloat16
F32 = mybir.dt.float32
bf16 = ml_dtypes.bfloat16

P = 128
C = 1024          # hidden
HG = 8            # heads per core
D = 64            # head dim
DG = HG * D       # 512, per-core qkv width
N_CORES = 8
FULL_T = 2048
SCALE = D ** -0.5


def build_kernel(T=FULL_T):
    nc = bacc.Bacc(
        "TRN2", target_bir_lowering=False, debug=False, num_devices=N_CORES
    )
    xs = [
        nc.dram_tensor(f"x{i}", [T // 4, C], BF16, kind="ExternalInput").ap()
        for i in range(4)
    ]
    wq = nc.dram_tensor("wq", [P, C // P, DG], BF16, kind="ExternalInput").ap()
    wk = nc.dram_tensor("wk", [P, C // P, DG], BF16, kind="ExternalInput").ap()
    wv = nc.dram_tensor("wv", [P, C // P, DG], BF16, kind="ExternalInput").ap()
    bq = nc.dram_tensor("bq", [P, 4], F32, kind="ExternalInput").ap()
    bv = nc.dram_tensor("bv", [1, DG], BF16, kind="ExternalInput").ap()
    pw = nc.dram_tensor("pw", [P, DG // P, C], BF16, kind="ExternalInput").ap()
    partial = nc.dram_tensor("partial", [T, C], F32, kind="ExternalOutput").ap()

    CT = C // P           # 8 contraction tiles over hidden
    TT = T // P           # 16 s-tiles of 128
    TCH = 512             # t chunk width
    NTH = T // TCH        # 4 t-chunks
    KT4 = DG // P         # 4 head-pairs (col-tiles of Q^T/K^T/O^T)

    with tile.TileContext(nc) as tc, ExitStack() as ctx:
        sb = ctx.enter_context(tc.tile_pool(name="sb", bufs=1))
        pdram = ctx.enter_context(tc.tile_pool(name="pdram", bufs=4, space="DRAM"))
        pon = ctx.enter_context(tc.tile_pool(name="pon", bufs=3))
        ppb = ctx.enter_context(tc.tile_pool(name="ppb", bufs=12))
        pout = ctx.enter_context(tc.tile_pool(name="pout", bufs=2))
        pp = ctx.enter_context(tc.tile_pool(name="pp", bufs=1, space="PSUM"))

        # persistent SBUF tensors
        xT = sb.tile([P, CT, T], BF16)
        wq_s = sb.tile([P, CT, DG], BF16)
        wk_s = sb.tile([P, CT, DG], BF16)
        wv_s = sb.tile([P, CT, DG], BF16)
        pw_s = sb.tile([P, KT4, C], BF16)
        bq_s = sb.tile([P, 4], F32)
        bv_s = sb.tile([1, DG], BF16)
        ones_s = sb.tile([1, P], BF16)
        QT = sb.tile([P, KT4, T], BF16)
        KTt = sb.tile([P, KT4, T], BF16)
        V = sb.tile([P, TT, HG, 65], BF16)
        OT = sb.tile([P, KT4, T], BF16)

        # ---- loads ----
        # transposes chunked by t so early attention groups only need the
        # first slices of x (input staging overlaps compute; x is split
        # into 4 input tensors interleaved with the weights in the
        # staging order)
        for sc in range(NTH):
            for ct in range(CT):
                nc.sync.dma_start(
                    out=xT[:, ct, sc * TCH : (sc + 1) * TCH],
                    in_=xs[sc][:, ct * P : (ct + 1) * P],
                    transpose=True,
                )
        # early weights on scalar queue; later ones on vector queue so no
        # staging-gated DMA instruction ever sits ahead of the EXPs in the
        # Scalar engine FIFO
        nc.scalar.dma_start(out=wk_s, in_=wk)
        nc.scalar.dma_start(out=wq_s, in_=wq)
        nc.scalar.dma_start(out=bq_s, in_=bq)
        nc.gpsimd.dma_start(out=wv_s, in_=wv)
        nc.gpsimd.dma_start(out=bv_s, in_=bv)
        nc.gpsimd.dma_start(out=pw_s, in_=pw)
        nc.vector.memset(ones_s, 1.0)
        nc.vector.memset(V[:, :, :, 64:65], 1.0)

        # ---- background units (each: a few PE matmuls + one DVE op) ----
        bg_ctr = [0]

        def bg_tile(nm):
            t = pp.tile([P, TCH], F32, tag=f"bg{bg_ctr[0] % 2}", name=nm)
            bg_ctr[0] += 1
            return t

        kt_done = [0] * KT4
        qt_done = set()

        def kt_unit(i, sc):
            ps = bg_tile("psk")
            for ct in range(CT):
                nc.tensor.matmul(
                    ps,
                    lhsT=wk_s[:, ct, i * P : (i + 1) * P],
                    rhs=xT[:, ct, sc * TCH : (sc + 1) * TCH],
                    start=(ct == 0),
                    stop=(ct == CT - 1),
                )
            nc.vector.tensor_copy(KTt[:, i, sc * TCH : (sc + 1) * TCH], ps)
            kt_done[i] += 1

        def qt_unit(i, th):
            ps = bg_tile("psq")
            for ct in range(CT):
                nc.tensor.matmul(
                    ps,
                    lhsT=wq_s[:, ct, i * P : (i + 1) * P],
                    rhs=xT[:, ct, th * TCH : (th + 1) * TCH],
                    start=(ct == 0),
                    stop=(ct == CT - 1),
                )
            nc.vector.tensor_scalar_add(
                QT[:, i, th * TCH : (th + 1) * TCH], ps, bq_s[:, i : i + 1]
            )
            qt_done.add((i, th))

        v_done = [0]

        def v_unit(tt):
            ps = bg_tile("psv")
            for ct in range(CT):
                nc.tensor.matmul(
                    ps,
                    lhsT=xT[:, ct, tt * P : (tt + 1) * P],
                    rhs=wv_s[:, ct, :],
                    start=(ct == 0),
                    stop=False,
                )
            nc.tensor.matmul(ps, lhsT=ones_s, rhs=bv_s, start=False, stop=True)
            nc.vector.tensor_copy(
                out=V[:, tt, :, 0:64],
                in_=ps.rearrange("p (h d) -> p h d", h=HG),
            )
            v_done[0] += 1

        def proj_unit(mt):
            ot = pout.tile([P, C], F32, tag="ot", name="ot")
            for nh in range(2):
                ps = bg_tile("psp")
                for kk in range(KT4):
                    nc.tensor.matmul(
                        ps,
                        lhsT=OT[:, kk, mt * P : (mt + 1) * P],
                        rhs=pw_s[:, kk, nh * TCH : (nh + 1) * TCH],
                        start=(kk == 0),
                        stop=(kk == KT4 - 1),
                    )
                nc.vector.tensor_copy(ot[:, nh * TCH : (nh + 1) * TCH], ps)
            eng = nc.sync if mt % 2 == 0 else nc.scalar
            eng.dma_start(out=partial[mt * P : (mt + 1) * P, :], in_=ot)

        bg = deque()
        bg_proj = deque()
        pending_muls = deque()
        pending_recips = deque()

        def drain(n=1):
            for _ in range(n):
                if bg:
                    bg.popleft()()

        def drain_until(pred):
            while bg and not pred():
                bg.popleft()()

        # ---- pre-attention: only what group (th0, hp0, st<4) needs ----
        # (x stages into DRAM progressively; kt(0, sc) chunks follow its
        # arrival order instead of blocking the first exp on all of x)
        kt_unit(0, 0)
        qt_unit(0, 0)

        # fill the background queue (order = rough priority; kt(0, sc)
        # interleaved early so S(st=4sc) unblocks as x chunks land)
        bg.append(lambda: v_unit(0))
        bg.append(lambda: v_unit(1))
        for sc in range(1, NTH):
            bg.append(lambda sc=sc: kt_unit(0, sc))
            bg.append(lambda tt=2 * sc: v_unit(tt))
            bg.append(lambda tt=2 * sc + 1: v_unit(tt))
        for tt in range(8, TT):
            bg.append(lambda tt=tt: v_unit(tt))
        for i in range(1, KT4):
            for sc in range(NTH):
                bg.append(lambda i=i, sc=sc: kt_unit(i, sc))
            bg.append(lambda i=i: qt_unit(i, 0))
        for i in range(KT4):
            bg.append(lambda i=i: qt_unit(i, 1))

        # ---- attention ----
        for th in range(NTH):
            for hp in range(KT4):
                gi = th * KT4 + hp  # group index
                # only the first kt chunk is needed to start; later chunks
                # are force-drained progressively inside the st loop so
                # the first S-pairs don't queue behind matmuls waiting on
                # not-yet-staged x chunks
                drain_until(
                    lambda hp=hp, th=th: kt_done[hp] >= 1 and (hp, th) in qt_done
                )

                def s_pair(st):
                    sw = pp.tile([P, 2 * TCH], F32, tag=f"sw{st % 2}", name="sw")
                    for hx in range(2):
                        pr = slice(hx * 64, (hx + 1) * 64)
                        nc.tensor.matmul(
                            sw[:, hx * TCH : (hx + 1) * TCH],
                            lhsT=KTt[pr, hp, st * P : (st + 1) * P],
                            rhs=QT[pr, hp, th * TCH : (th + 1) * TCH],
                            start=True,
                            stop=True,
                        )
                    return sw

                o_t = [
                    pp.tile([65, TCH], F32, tag=("oA", "oB")[hx], name="o_t")
                    for hx in range(2)
                ]
                sws = [s_pair(0), s_pair(1)]
                for st in range(TT):
                    pb = ppb.tile([P, 2 * TCH], BF16, tag="pb", name="pb")
                    nc.scalar.activation(
                        out=pb,
                        in_=sws[st % 2],
                        func=mybir.ActivationFunctionType.Exp,
                        scale=float(SCALE),
                    )
                    if st + 2 < TT:
                        drain_until(
                            lambda hp=hp, need=(st + 2) // 4 + 1: kt_done[hp]
                            >= need
                        )
                        sws[st % 2] = s_pair(st + 2)
                    # V(st) must be EMITTED before PV(st) reads it
                    drain_until(lambda st=st: v_done[0] > st)
                    for hx in range(2):
                        nc.tensor.matmul(
                            o_t[hx],
                            lhsT=V[:, st, 2 * hp + hx, :],
                            rhs=pb[:, hx * TCH : (hx + 1) * TCH],
                            start=(st == 0),
                            stop=(st == TT - 1),
                        )
                    drain(2 if len(bg) > 8 else 1)
                    # previous group's reciprocal packages, spread so the
                    # 3.3us DVE reciprocals never collide with PE-critical
                    # DVE work (proj copies pop at st>=7, after them)
                    if st in (2, 4) and pending_recips:
                        pending_recips.popleft()()
                    if st == 6:
                        while pending_muls and pending_muls[0][0] <= gi - 2:
                            pending_muls.popleft()[1]()
                    if not bg and bg_proj and bg_proj[0][0] <= gi and st >= 7:
                        bg_proj.popleft()[1]()

                # ---- normalize this group's output (deferred chain) ----
                # only the PSUM-freeing ou copies happen here; the
                # reciprocal+bounce+broadcast packages run early in the
                # NEXT group and the muls one group after that
                ou = []
                for hx in range(2):
                    o = pon.tile([65, TCH], F32, tag=f"ou{hx}", name="ou")
                    nc.vector.tensor_copy(o, o_t[hx])
                    ou.append(o)
                rid = pdram.tile([2, TCH], F32, tag="rid", name="rid")

                def recip_pkg(hx, ou=ou, rid=rid, gi=gi, hp=hp, th=th):
                    rr = pon.tile([1, TCH], F32, tag=f"rr{hx}", name="rr")
                    nc.vector.reciprocal(rr, ou[hx][64:65, :])
                    nc.gpsimd.dma_start(out=rid[hx : hx + 1, :], in_=rr)
                    rb = pon.tile([64, TCH], F32, tag=f"rb{hx}", name="rb")
                    nc.gpsimd.dma_start(
                        out=rb, in_=rid[hx : hx + 1, :].to_broadcast((64, TCH))
                    )

                    def mul():
                        nc.vector.tensor_mul(
                            OT[
                                hx * 64 : (hx + 1) * 64,
                                hp,
                                th * TCH : (th + 1) * TCH,
                            ],
                            ou[hx][0:64, :],
                            rb,
                        )

                    pending_muls.append((gi, mul))

                pending_recips.append(lambda: recip_pkg(0))
                pending_recips.append(lambda: recip_pkg(1))

            # after all head-pairs of this t-chunk: projection for its tiles
            for mt in range(th * (TCH // P), (th + 1) * (TCH // P)):
                bg_proj.append((th * KT4 + 6, lambda mt=mt: proj_unit(mt)))
            if th + 2 < NTH:
                for i in range(KT4):
                    bg.append(lambda i=i, th=th: qt_unit(i, th + 2))

        while pending_recips:
            pending_recips.popleft()()
        while pending_muls:
            pending_muls.popleft()[1]()
        drain(len(bg))
        while bg_proj:
            bg_proj.popleft()[1]()

    nc.compile()
    return nc


def shard_inputs(x, qkv_w, qkv_b, proj_w, proj_b, T=FULL_T):
    """Build the 8 per-core input maps (host-side layout prep)."""
    x = np.asarray(x, dtype=np.float32)
    qkv_w = np.asarray(qkv_w, dtype=np.float32)
    qkv_b = np.asarray(qkv_b, dtype=np.float32)
    proj_w = np.asarray(proj_w, dtype=np.float32)
    in_maps = []
    for c in range(N_CORES):
        b, g = divmod(c, 2)
        sl = slice(g * DG, (g + 1) * DG)
        wqg = qkv_w[:, 0 * C + g * DG : 0 * C + (g + 1) * DG]
        wkg = qkv_w[:, 1 * C + g * DG : 1 * C + (g + 1) * DG]
        wvg = qkv_w[:, 2 * C + g * DG : 2 * C + (g + 1) * DG]
        bqg = qkv_b[0 * C + g * DG : 0 * C + (g + 1) * DG]
        bvg = qkv_b[2 * C + g * DG : 2 * C + (g + 1) * DG]
        pwg = proj_w[sl, :]

        def arr_w(w):  # [C, DG] -> [128, C//128, DG]
            return np.ascontiguousarray(
                w.reshape(C // P, P, DG).transpose(1, 0, 2)
            ).astype(bf16)

        xb = np.ascontiguousarray(x[b, :T]).astype(bf16)
        q = T // 4
        in_maps.append(
            {
                # dict order ~ staging order: first-needed tensors first,
                # x split into chunks so weights interleave with it
                "wk": arr_w(wkg),
                "wq": arr_w(wqg),
                "bq": np.ascontiguousarray(
                    bqg.reshape(DG // P, P).T
                ).astype(np.float32),
                "x0": xb[0:q],
                "x1": xb[q : 2 * q],
                "wv": arr_w(wvg),
                "x2": xb[2 * q : 3 * q],
                "bv": np.ascontiguousarray(bvg[None, :]).astype(bf16),
                "x3": xb[3 * q : 4 * q],
                "pw": np.ascontiguousarray(
                    pwg.reshape(DG // P, P, C).transpose(1, 0, 2)
                ).astype(bf16),
            }
        )
    return in_maps


def combine_outputs(results, proj_b, T=FULL_T):
    proj_b = np.asarray(proj_b, dtype=np.float32)
    out = np.empty((N_CORES // 2, T, C), np.float32)
    for b in range(N_CORES // 2):
        out[b] = (
            results[2 * b]["partial"] + results[2 * b + 1]["partial"] + proj_b
        )
    return out


_NC_CACHE = {}


def _get_nc(T=FULL_T):
    if T not in _NC_CACHE:
        _NC_CACHE[T] = build_kernel(T)
    return _NC_CACHE[T]


def run(x, qkv_w, qkv_b, proj_w, proj_b, trace=False):
    nc = _get_nc()
    in_maps = shard_inputs(x, qkv_w, qkv_b, proj_w, proj_b)
    res = run_bass_kernel_spmd(nc, in_maps, list(range(N_CORES)), trace=trace)
    return combine_outputs(res.results, proj_b), res


def kernel(x, qkv_w, qkv_b, proj_w, proj_b):
    out, _ = run(x, qkv_w, qkv_b, proj_w, proj_b)
    return out


# revision 48
# speedup vs baseline: 1.0159x; 1.0159x over previous
"""Multi-head self-attention TRN2 kernel (v2).

Full inputs -> shard over 8 NeuronCores as (batch b, head-group g):
core c = 2*b + g handles batch b and heads 8g..8g+7. Each core computes its
heads' contribution to the output projection; the host sums the two partials
per batch and adds proj bias.

v2 structure (per core, T=2048, C=1024, 4 head-pairs, t-chunks of 512):
  - One [128, 1024] PSUM S-window per (st) step holds BOTH heads of the
    pair -> a single EXP instruction per step (ScalarE is the pacer).
  - The two K=64 score matmuls are emitted back-to-back with row groups
    0-63 / 64-127 so they execute concurrently in the PE array.
  - PSUM = 2 double-buffered S-windows (4 banks) + 2 O accumulators
    (2 banks, 65 rows: d 0-63 + rowsum via ones-column of V_aug) + 2
    background banks for the QKV/proj streams. 8 banks exactly.
  - QKV/proj matmuls are "background units" drained one per st step into
    the PE slack under EXP; attention starts as soon as K^T(hp0) and
    Q^T(hp0, th0) exist.
  - Softmax denominators: rowsum rows are bounced to DRAM, regathered as
    [128, 8] (t-major) for one cheap DVE reciprocal, scattered back, then
    stride-0 broadcast to [64, 512] for the normalize multiply.
  - K bias is dropped entirely: adding bk shifts every score column by a
    t-constant, which softmax cancels. Only Q needs its bias.
"""

import numpy as np
import ml_dtypes
from collections import deque
from contextlib import ExitStack

import concourse.bass as bass
import concourse.bacc as bacc
import concourse.tile as tile
from concourse import mybir
from concourse.bass_utils import run_bass_kernel_spmd

BF16 = mybir.dt.bfloat16
F32 = mybir.dt.float32
bf16 = ml_dtypes.bfloat16

P = 128
C = 1024          # hidden
HG = 8            # heads per core
D = 64            # head dim
DG = HG * D       # 512, per-core qkv width
N_CORES = 8
FULL_T = 2048
SCALE = D ** -0.5


def build_kernel(T=FULL_T):
    nc = bacc.Bacc(
        "TRN2", target_bir_lowering=False, debug=False, num_devices=N_CORES
    )
    xs = [
        nc.dram_tensor(f"x{i}", [T // 4, C], BF16, kind="ExternalInput").ap()
        for i in range(4)
    ]
    wq = nc.dram_tensor("wq", [P, C // P, DG], BF16, kind="ExternalInput").ap()
    wk = nc.dram_tensor("wk", [P, C // P, DG], BF16, kind="ExternalInput").ap()
    wv = nc.dram_tensor("wv", [P, C // P, DG], BF16, kind="ExternalInput").ap()
    bq = nc.dram_tensor("bq", [P, 4], F32, kind="ExternalInput").ap()
    bv = nc.dram_tensor("bv", [1, DG], BF16, kind="ExternalInput").ap()
    pw = nc.dram_tensor("pw", [P, DG // P, C], BF16, kind="ExternalInput").ap()
    partial = nc.dram_tensor("partial", [T, C], F32, kind="ExternalOutput").ap()

    CT = C // P           # 8 contraction tiles over hidden
    TT = T // P           # 16 s-tiles of 128
    TCH = 512             # t chunk width
    NTH = T // TCH        # 4 t-chunks
    KT4 = DG // P         # 4 head-pairs (col-tiles of Q^T/K^T/O^T)

    with tile.TileContext(nc) as tc, ExitStack() as ctx:
        sb = ctx.enter_context(tc.tile_pool(name="sb", bufs=1))
        pdram = ctx.enter_context(tc.tile_pool(name="pdram", bufs=4, space="DRAM"))
        pon = ctx.enter_context(tc.tile_pool(name="pon", bufs=3))
        ppb = ctx.enter_context(tc.tile_pool(name="ppb", bufs=12))
        pout = ctx.enter_context(tc.tile_pool(name="pout", bufs=2))
        pp = ctx.enter_context(tc.tile_pool(name="pp", bufs=1, space="PSUM"))

        # persistent SBUF tensors
        xT = sb.tile([P, CT, T], BF16)
        wq_s = sb.tile([P, CT, DG], BF16)
        wk_s = sb.tile([P, CT, DG], BF16)
        wv_s = sb.tile([P, CT, DG], BF16)
        pw_s = sb.tile([P, KT4, C], BF16)
        bq_s = sb.tile([P, 4], F32)
        bv_s = sb.tile([1, DG], BF16)
        ones_s = sb.tile([1, P], BF16)
        QT = sb.tile([P, KT4, T], BF16)
        KTt = sb.tile([P, KT4, T], BF16)
        V = sb.tile([P, TT, HG, 65], BF16)
        OT = sb.tile([P, KT4, T], BF16)

        # ---- loads ----
        # transposes chunked by t so early attention groups only need the
        # first slices of x (input staging overlaps compute; x is split
        # into 4 input tensors interleaved with the weights in the
        # staging order)
        for sc in range(NTH):
            for ct in range(CT):
                nc.sync.dma_start(
                    out=xT[:, ct, sc * TCH : (sc + 1) * TCH],
                    in_=xs[sc][:, ct * P : (ct + 1) * P],
                    transpose=True,
                )
        # early weights on scalar queue; later ones on vector queue so no
        # staging-gated DMA instruction ever sits ahead of the EXPs in the
        # Scalar engine FIFO
        nc.scalar.dma_start(out=wk_s, in_=wk)
        nc.scalar.dma_start(out=wq_s, in_=wq)
        nc.scalar.dma_start(out=bq_s, in_=bq)
        nc.gpsimd.dma_start(out=wv_s, in_=wv)
        nc.gpsimd.dma_start(out=bv_s, in_=bv)
        nc.gpsimd.dma_start(out=pw_s, in_=pw)
        nc.vector.memset(ones_s, 1.0)
        nc.vector.memset(V[:, :, :, 64:65], 1.0)

        # ---- background units (each: a few PE matmuls + one DVE op) ----
        bg_ctr = [0]

        def bg_tile(nm):
            t = pp.tile([P, TCH], F32, tag=f"bg{bg_ctr[0] % 2}", name=nm)
            bg_ctr[0] += 1
            return t

        kt_done = [0] * KT4
        qt_done = set()

        def kt_unit(i, sc):
            ps = bg_tile("psk")
            for ct in range(CT):
                nc.tensor.matmul(
                    ps,
                    lhsT=wk_s[:, ct, i * P : (i + 1) * P],
                    rhs=xT[:, ct, sc * TCH : (sc + 1) * TCH],
                    start=(ct == 0),
                    stop=(ct == CT - 1),
                )
            nc.vector.tensor_copy(KTt[:, i, sc * TCH : (sc + 1) * TCH], ps)
            kt_done[i] += 1

        def qt_unit(i, th):
            ps = bg_tile("psq")
            for ct in range(CT):
                nc.tensor.matmul(
                    ps,
                    lhsT=wq_s[:, ct, i * P : (i + 1) * P],
                    rhs=xT[:, ct, th * TCH : (th + 1) * TCH],
                    start=(ct == 0),
                    stop=(ct == CT - 1),
                )
            nc.vector.tensor_scalar_add(
                QT[:, i, th * TCH : (th + 1) * TCH], ps, bq_s[:, i : i + 1]
            )
            qt_done.add((i, th))

        v_done = [0]

        def v_unit(tt):
            ps = bg_tile("psv")
            for ct in range(CT):
                nc.tensor.matmul(
                    ps,
                    lhsT=xT[:, ct, tt * P : (tt + 1) * P],
                    rhs=wv_s[:, ct, :],
                    start=(ct == 0),
                    stop=False,
                )
            nc.tensor.matmul(ps, lhsT=ones_s, rhs=bv_s, start=False, stop=True)
            nc.vector.tensor_copy(
                out=V[:, tt, :, 0:64],
                in_=ps.rearrange("p (h d) -> p h d", h=HG),
            )
            v_done[0] += 1

        def proj_unit(mt):
            ot = pout.tile([P, C], F32, tag="ot", name="ot")
            for nh in range(2):
                ps = bg_tile("psp")
                for kk in range(KT4):
                    nc.tensor.matmul(
                        ps,
                        lhsT=OT[:, kk, mt * P : (mt + 1) * P],
                        rhs=pw_s[:, kk, nh * TCH : (nh + 1) * TCH],
                        start=(kk == 0),
                        stop=(kk == KT4 - 1),
                    )
                nc.vector.tensor_copy(ot[:, nh * TCH : (nh + 1) * TCH], ps)
            eng = nc.sync if mt % 2 == 0 else nc.scalar
            eng.dma_start(out=partial[mt * P : (mt + 1) * P, :], in_=ot)

        bg = deque()
        bg_proj = deque()
        pending_muls = deque()
        pending_recips = deque()

        def drain(n=1):
            for _ in range(n):
                if bg:
                    bg.popleft()()

        def drain_until(pred):
            while bg and not pred():
                bg.popleft()()

        # ---- pre-attention: only what group (th0, hp0, st<4) needs ----
        # (x stages into DRAM progressively; kt(0, sc) chunks follow its
        # arrival order instead of blocking the first exp on all of x)
        kt_unit(0, 0)
        qt_unit(0, 0)

        # fill the background queue (order = rough priority; kt(0, sc)
        # interleaved early so S(st=4sc) unblocks as x chunks land)
        bg.append(lambda: v_unit(0))
        bg.append(lambda: v_unit(1))
        for sc in range(1, NTH):
            bg.append(lambda sc=sc: kt_unit(0, sc))
            bg.append(lambda tt=2 * sc: v_unit(tt))
            bg.append(lambda tt=2 * sc + 1: v_unit(tt))
        for tt in range(8, TT):
            bg.append(lambda tt=tt: v_unit(tt))
        for i in range(1, KT4):
            for sc in range(NTH):
                bg.append(lambda i=i, sc=sc: kt_unit(i, sc))
            bg.append(lambda i=i: qt_unit(i, 0))
        for i in range(KT4):
            bg.append(lambda i=i: qt_unit(i, 1))

        # ---- attention ----
        for th in range(NTH):
            for hp in range(KT4):
                gi = th * KT4 + hp  # group index
                # only the first kt chunk is needed to start; later chunks
                # are force-drained progressively inside the st loop so
                # the first S-pairs don't queue behind matmuls waiting on
                # not-yet-staged x chunks
                drain_until(
                    lambda hp=hp, th=th: kt_done[hp] >= 1 and (hp, th) in qt_done
                )

                def s_pair(st):
                    sw = pp.tile([P, 2 * TCH], F32, tag=f"sw{st % 2}", name="sw")
                    for hx in range(2):
                        pr = slice(hx * 64, (hx + 1) * 64)
                        nc.tensor.matmul(
                            sw[:, hx * TCH : (hx + 1) * TCH],
                            lhsT=KTt[pr, hp, st * P : (st + 1) * P],
                            rhs=QT[pr, hp, th * TCH : (th + 1) * TCH],
                            start=True,
                            stop=True,
                        )
                    return sw

                o_t = [
                    pp.tile([65, TCH], F32, tag=("oA", "oB")[hx], name="o_t")
                    for hx in range(2)
                ]
                sws = [s_pair(0), s_pair(1)]
                for st in range(TT):
                    pb = ppb.tile([P, 2 * TCH], BF16, tag="pb", name="pb")
                    nc.scalar.activation(
                        out=pb,
                        in_=sws[st % 2],
                        func=mybir.ActivationFunctionType.Exp,
                        scale=float(SCALE),
                    )
                    if st + 2 < TT:
                        drain_until(
                            lambda hp=hp, need=(st + 2) // 4 + 1: kt_done[hp]
                            >= need
                        )
                        sws[st % 2] = s_pair(st + 2)
                    # V(st) must be EMITTED before PV(st) reads it
                    drain_until(lambda st=st: v_done[0] > st)
                    for hx in range(2):
                        nc.tensor.matmul(
                            o_t[hx],
                            lhsT=V[:, st, 2 * hp + hx, :],
                            rhs=pb[:, hx * TCH : (hx + 1) * TCH],
                            start=(st == 0),
                            stop=(st == TT - 1),
                        )
                    drain(2 if len(bg) > 8 else 1)
                    # previous group's reciprocal work, chopped into
                    # [1,128] chunks popped one per step: a DVE copy
                    # queued behind a chunk waits <1us instead of ~7us
                    # (bg-bank WAR couples DVE copy delays into the PE
                    # FIFO, so big DVE bursts stall the next group)
                    if st >= 1 and pending_recips:
                        pending_recips.popleft()()
                    if st == 6:
                        while pending_muls and pending_muls[0][0] <= gi - 2:
                            pending_muls.popleft()[1]()
                    if not bg and bg_proj and bg_proj[0][0] <= gi and st >= 7:
                        bg_proj.popleft()[1]()

                # ---- normalize this group's output (deferred chain) ----
                # only the PSUM-freeing ou copies happen here; the
                # reciprocal+bounce+broadcast packages run early in the
                # NEXT group and the muls one group after that
                ou = []
                for hx in range(2):
                    o = pon.tile([65, TCH], F32, tag=f"ou{hx}", name="ou")
                    nc.vector.tensor_copy(o, o_t[hx])
                    ou.append(o)
                rid = pdram.tile([2, TCH], F32, tag="rid", name="rid")

                rrs = [
                    pon.tile([1, TCH], F32, tag=f"rr{hx}", name="rr")
                    for hx in range(2)
                ]

                def recip_chunk(hx, c, ou=ou, rrs=rrs):
                    nc.vector.reciprocal(
                        rrs[hx][:, c * P : (c + 1) * P],
                        ou[hx][64:65, c * P : (c + 1) * P],
                    )

                def recip_fin(hx, ou=ou, rrs=rrs, rid=rid, gi=gi, hp=hp, th=th):
                    nc.gpsimd.dma_start(out=rid[hx : hx + 1, :], in_=rrs[hx])
                    rb = pon.tile([64, TCH], F32, tag=f"rb{hx}", name="rb")
                    nc.gpsimd.dma_start(
                        out=rb, in_=rid[hx : hx + 1, :].to_broadcast((64, TCH))
                    )

                    def mul():
                        nc.vector.tensor_mul(
                            OT[
                                hx * 64 : (hx + 1) * 64,
                                hp,
                                th * TCH : (th + 1) * TCH,
                            ],
                            ou[hx][0:64, :],
                            rb,
                        )

                    pending_muls.append((gi, mul))

                for hx in range(2):
                    for c in range(TCH // P):
                        pending_recips.append(
                            lambda hx=hx, c=c: recip_chunk(hx, c)
                        )
                    pending_recips.append(lambda hx=hx: recip_fin(hx))

            # after all head-pairs of this t-chunk: projection for its tiles
            for mt in range(th * (TCH // P), (th + 1) * (TCH // P)):
                bg_proj.append((th * KT4 + 6, lambda mt=mt: proj_unit(mt)))
            if th + 2 < NTH:
                for i in range(KT4):
                    bg.append(lambda i=i, th=th: qt_unit(i, th + 2))

        while pending_recips:
            pending_recips.popleft()()
        while pending_muls:
            pending_muls.popleft()[1]()
        drain(len(bg))
        while bg_proj:
            bg_proj.popleft()[1]()

    nc.compile()
    return nc


def shard_inputs(x, qkv_w, qkv_b, proj_w, proj_b, T=FULL_T):
    """Build the 8 per-core input maps (host-side layout prep)."""
    x = np.asarray(x, dtype=np.float32)
    qkv_w = np.asarray(qkv_w, dtype=np.float32)
    qkv_b = np.asarray(qkv_b, dtype=np.float32)
    proj_w = np.asarray(proj_w, dtype=np.float32)
    in_maps = []
    for c in range(N_CORES):
        b, g = divmod(c, 2)
        sl = slice(g * DG, (g + 1) * DG)
        wqg = qkv_w[:, 0 * C + g * DG : 0 * C + (g + 1) * DG]
        wkg = qkv_w[:, 1 * C + g * DG : 1 * C + (g + 1) * DG]
        wvg = qkv_w[:, 2 * C + g * DG : 2 * C + (g + 1) * DG]
        bqg = qkv_b[0 * C + g * DG : 0 * C + (g + 1) * DG]
        bvg = qkv_b[2 * C + g * DG : 2 * C + (g + 1) * DG]
        pwg = proj_w[sl, :]

        def arr_w(w):  # [C, DG] -> [128, C//128, DG]
            return np.ascontiguousarray(
                w.reshape(C // P, P, DG).transpose(1, 0, 2)
            ).astype(bf16)

        xb = np.ascontiguousarray(x[b, :T]).astype(bf16)
        q = T // 4
        in_maps.append(
            {
                # dict order ~ staging order: first-needed tensors first,
                # x split into chunks so weights interleave with it
                "wk": arr_w(wkg),
                "wq": arr_w(wqg),
                "bq": np.ascontiguousarray(
                    bqg.reshape(DG // P, P).T
                ).astype(np.float32),
                "x0": xb[0:q],
                "x1": xb[q : 2 * q],
                "wv": arr_w(wvg),
                "x2": xb[2 * q : 3 * q],
                "bv": np.ascontiguousarray(bvg[None, :]).astype(bf16),
                "x3": xb[3 * q : 4 * q],
                "pw": np.ascontiguousarray(
                    pwg.reshape(DG // P, P, C).transpose(1, 0, 2)
                ).astype(bf16),
            }
        )
    return in_maps


def combine_outputs(results, proj_b, T=FULL_T):
    proj_b = np.asarray(proj_b, dtype=np.float32)
    out = np.empty((N_CORES // 2, T, C), np.float32)
    for b in range(N_CORES // 2):
        out[b] = (
            results[2 * b]["partial"] + results[2 * b + 1]["partial"] + proj_b
        )
    return out


_NC_CACHE = {}


def _get_nc(T=FULL_T):
    if T not in _NC_CACHE:
        _NC_CACHE[T] = build_kernel(T)
    return _NC_CACHE[T]


def run(x, qkv_w, qkv_b, proj_w, proj_b, trace=False):
    nc = _get_nc()
    in_maps = shard_inputs(x, qkv_w, qkv_b, proj_w, proj_b)
    res = run_bass_kernel_spmd(nc, in_maps, list(range(N_CORES)), trace=trace)
    return combine_outputs(res.results, proj_b), res


def kernel(x, qkv_w, qkv_b, proj_w, proj_b):
    out, _ = run(x, qkv_w, qkv_b, proj_w, proj_b)
    return out
